# revision 33
# baseline (speedup 1.0000x reference)
"""Causal single-head attention on 8 Trainium2 NeuronCores.

Problem: x [4, 2048, 1024] f32; Wq/Wk/Wv [1024, 1024] f32.
  q,k,v = x@W*; out = softmax(causal(q k^T / sqrt(d))) @ v.

Two algebraic folds remove all cross-core communication:

1. scores = (x Wq)(x Wk)^T = x (Wq Wk^T) x^T. The host precomputes
   M = Wq Wk^T once (weight-only preprocessing), so the device computes
   q~ = x M and uses raw x^T as the key matrix — the whole k projection
   and any k exchange disappear.
2. att @ v = att @ (x Wv) = ((x^T E)^T Wv): the device computes
   U^T[d, q] = x^T E (contracting keys) and then out = (U Wv)/denom.
   Same tensor-engine row count as v-proj + att@v, but v never needs to
   be materialized. The kernel has NO collectives; every operand is a
   direct per-core input.

Sharding: 8 cores = 4 batches x 2 query-interleavings. Each core owns
four 256-query blocks chosen so the causal key-prefix lengths fit the
shared ascending slot shape (4, 8, 12, 16) x 128-key tiles with only 4
padded tiles per core (exact fold balance; SPMD: one program, all
cores). Blocks run smallest-first so the first block needs the least
input data, and are processed as PAIRS (0,1) and (2,3): the pair's two
query blocks sit contiguously in qt/us, so scores and U^T run as joint
N=512 chains over the pair's shared key tiles and narrow N=256 chains
for the larger block's extra tiles (wider matmuls stream ~8% more
rows/cycle and halve instruction count).

Causal masking is generated ON DEVICE: one iota row constant plus a
per-tile threshold column (~13 KB total DMA) expand to the additive
-60000 mask via a fused vector compare-multiply, replacing 3.1 MB of
host mask DMA that would rate-limit the scores phase.

DMA scheduling: HBM bandwidth is shared fairly per inflight TRANSFER,
so issue order is everything. Wave 1 (issued at program start, 5 small
transfers across the sync/scalar/gpsimd queues) carries only the
q~-projection working set; every other input is issued by a dma_start
anchored behind a specific scalar-engine compute op, so it cannot
enter flight early and steal bandwidth from earlier-needed bytes. Each
anchored DMA must still precede its first consumer in PROGRAM order —
an anchor after the consumer inverts the dependency into
read-before-write garbage.

All matmul operands are fp16 (1 PE cycle/row; fp32 is 4x slower).
Accumulation stays fp32 in PSUM throughout. PSUM start/stop flags are
managed per column REGION of the accumulator (legal: regions are
independent element-wise RMW state), which lets the joint U^T chain
close its left half at the small block's last key tile while the right
half keeps accumulating.

Per-core dataflow per block pair (joint width 512, solo width 256):
  q~T[j, q] = M^T x_q^T               (one 512-wide pass)
  scores S^T[k, q] = x-block q~T      (accum over 8 j-chunks)
  E = exp((S^T + mask) / 32)          (ACT, fp16 out)
  U^T[d, q] = x-rows^T E              (accum over key tiles)
  d[q] = E^T 1                        (N=2 ones-matmul per block)
  out[q, e] = (U^T^T Wv) * (1/d)      (accum over 8 d-chunks)

Softmax max-subtraction is skipped deliberately: logits*scale are
bounded, so exp is well-conditioned.
"""

import os
import sys
from contextlib import ExitStack

sys.path.insert(0, "/opt/trn_rl_repo")

import numpy as np

import concourse.bass as bass  # noqa: F401
import concourse.tile as tile
from concourse import bacc, mybir
from concourse.bass_utils import run_bass_kernel_spmd

B, T, D = 4, 2048, 1024
P = 128                 # partitions
DC = D // P             # 8 contraction chunks
QB = 256                # queries per block
NB = 4                  # blocks per core
NQ = QB * NB            # 1024 queries per core
SLOTS = (4, 8, 12, 16)  # 128-key tiles per block slot (ascending)
NKT = sum(SLOTS)        # 40
TCH = T // P            # 16 key 128-token chunks
XRC = 4                 # x-row DMA chunks (4 key tiles each)
NG = 4                  # x^T DMA groups (4 key tiles each)
SCALE = 1.0 / 32.0      # 1/sqrt(D)
MASK_NEG = -60000.0

# query-block start per (half, slot position); slot order ascending
QLOS = ((0, 768, 1024, 1792),      # even cores
        (256, 512, 1280, 1536))    # odd cores

F16 = mybir.dt.float16
F32 = mybir.dt.float32

_CACHE = {}

last_exec_time_ns = None  # set when BASS_KERNEL_TRACE=1


def _build_program():
    nc = bacc.Bacc("TRN2", target_bir_lowering=False, debug=False, num_devices=8)

    xq_d = nc.dram_tensor("xq", [2, P, DC, 512], F16, kind="ExternalInput")
    mm_d = nc.dram_tensor("mm", [P, DC, DC, P], F16, kind="ExternalInput")
    xt_d = nc.dram_tensor("xt", [NG, P, DC, 512], F16, kind="ExternalInput")
    xr_d = nc.dram_tensor("xr", [XRC, P, 4, D], F16, kind="ExternalInput")
    wv_d = nc.dram_tensor("wv", [2, P, DC, 512], F16, kind="ExternalInput")
    iota_d = nc.dram_tensor("iota", [P, QB], F32, kind="ExternalInput")
    thr_d = nc.dram_tensor("thr", [P, NKT], F32, kind="ExternalInput")
    out_d = nc.dram_tensor("out", [2, NQ, 512], F32, kind="ExternalOutput")

    with tile.TileContext(nc) as tc, ExitStack() as stack:
        p_mm = stack.enter_context(tc.tile_pool(name="mm", bufs=1))
        p_xq = stack.enter_context(tc.tile_pool(name="xq", bufs=1))
        p_xt = stack.enter_context(tc.tile_pool(name="xt", bufs=1))
        p_xr = stack.enter_context(tc.tile_pool(name="xr", bufs=1))
        p_wv = stack.enter_context(tc.tile_pool(name="wv", bufs=1))
        p_qt = stack.enter_context(tc.tile_pool(name="qt", bufs=2))
        p_us = stack.enter_context(tc.tile_pool(name="us", bufs=2))
        p_e = stack.enter_context(tc.tile_pool(name="e", bufs=1))
        p_misc = stack.enter_context(tc.tile_pool(name="misc", bufs=1))
        p_mk = stack.enter_context(tc.tile_pool(name="mk", bufs=2))
        p_sm = stack.enter_context(tc.tile_pool(name="sm", bufs=2))
        p_out = stack.enter_context(tc.tile_pool(name="outp", bufs=3))
        ps_a = stack.enter_context(tc.tile_pool(name="psa", bufs=2, space="PSUM"))
        ps_b = stack.enter_context(tc.tile_pool(name="psb", bufs=3, space="PSUM"))
        psd = stack.enter_context(tc.tile_pool(name="psd", bufs=2, space="PSUM"))
        if True:
            # ---- tiny constants (scalar queue, ahead of everything) ----
            iota_t = p_misc.tile([P, QB], F32, tag="iota")
            nc.scalar.dma_start(iota_t[:], iota_d.ap())
            thr_t = p_misc.tile([P, NKT], F32, tag="thr")
            nc.scalar.dma_start(thr_t[:], thr_d.ap())
            ones_t = p_misc.tile([P, 2], F16, tag="ones")
            nc.vector.memset(ones_t[:], 1.0)

            # ---- input loads: explicit need-ordered queue assignment.
            xt_g = []
            xr_c = [None] * XRC

            # Allocate all input tiles up front; DMA issue is staged.
            xqA = p_xq.tile([P, DC, 512], F16, tag="xq0")
            xqB = p_xq.tile([P, DC, 512], F16, tag="xq1")
            xq_p = [xqA, xqB]
            mm_t = p_mm.tile([P, DC, DC, P], F16, tag="mm")
            for g in range(NG):
                xg = p_xt.tile([P, DC, 512], F16, tag=f"xt{g}")
                xt_g.append(xg)
            for c in range(XRC):
                xc = p_xr.tile([P, 4, D], F16, tag=f"xr{c}")
                xr_c[c] = xc
            wv0_t = p_wv.tile([P, DC, 512], F16, tag="wv0")
            wv1_t = p_wv.tile([P, DC, 512], F16, tag="wv1")
            wv_h = [wv0_t, wv1_t]

            # Immediate issues — ONLY the q~ pair-A working set. HBM
            # splits bandwidth per inflight TRANSFER, so the first wave
            # must be as few transfers as possible (5 here) for the
            # first-needed bytes to land fast.
            nc.sync.dma_start(xqA[:, 0:4], xq_d.ap()[0][:, 0:4])
            nc.gpsimd.dma_start(xqA[:, 4:8], xq_d.ap()[0][:, 4:8])
            nc.scalar.dma_start(mm_t[:, 0:2], mm_d.ap()[:, 0:2])
            nc.scalar.dma_start(mm_t[:, 2:4], mm_d.ap()[:, 2:4])
            nc.scalar.dma_start(mm_t[:, 4:8], mm_d.ap()[:, 4:8])

            # Deferred issues, anchored behind scalar compute ops so
            # they cannot enter flight before the anchor executes —
            # each wave stays small and lands well before its consumer.
            defer = {
                ("qt", 0, 0): (xt_g[0][:], xt_d.ap()[0]),
                ("qt", 0, 1): (xr_c[0][:], xr_d.ap()[0]),
                ("qt", 0, 4): (wv0_t[:], wv_d.ap()[0]),
                ("qt", 0, 6): (wv1_t[:], wv_d.ap()[1]),
                ("exp", 0, 1): (xt_g[1][:], xt_d.ap()[1]),
                ("exp", 0, 3): (xr_c[1][:], xr_d.ap()[1]),
                ("us", 0, 3): (xqB[:], xq_d.ap()[1]),
                ("exp", 1, 3): (xt_g[2][:], xt_d.ap()[2]),
                ("us", 0, 6): (xr_c[2][:], xr_d.ap()[2]),
                ("exp", 2, 1): (xt_g[3][:], xt_d.ap()[3]),
                ("exp", 2, 3): (xr_c[3][:], xr_d.ap()[3]),
            }

            def anchor(key):
                if key in defer:
                    dst, src = defer.pop(key)
                    nc.scalar.dma_start(dst, src)

            # ---- per-pair pipeline: blocks (2p, 2p+1) share key tiles
            # and sit contiguously in qt, so scores and U^T run as
            # joint N=512 chains over the shared key range and narrow
            # N=256 chains for the larger block's extra tiles.
            for pair in range(NB // 2):
                bA, bB = 2 * pair, 2 * pair + 1
                nA, nB_ = SLOTS[bA], SLOTS[bB]
                baseA = sum(SLOTS[:bA])
                baseB = sum(SLOTS[:bB])

                # q~ projection, one 512-wide pass per pair
                qt_t = p_qt.tile([P, DC, 512], F16, tag="qt")
                for jc in range(DC):
                    acc = ps_b.tile([P, 512], F32, tag="psb")
                    for dc in range(DC):
                        nc.tensor.matmul(acc[:],
                                         mm_t[:, jc, dc, :],
                                         xq_p[pair][:, dc, :],
                                         start=(dc == 0),
                                         stop=(dc == DC - 1))
                    nc.scalar.copy(qt_t[:, jc, :], acc[:])
                    anchor(("qt", bA, jc))

                # scores + on-device causal mask + exp.
                # kt < nA: joint N=512 tile for both blocks; the mask
                # (per-block threshold on each half) restores causality.
                e_t = p_e.tile([P, nB_, 512], F16, tag=f"e{pair}")
                for kt in range(nB_):
                    joint = kt < nA
                    co = 0 if joint else QB
                    w = 512 - co
                    acc = (ps_b if joint else ps_a).tile(
                        [P, w], F32, tag="psb" if joint else "psa")
                    for jc in range(DC):
                        nc.tensor.matmul(
                            acc[:],
                            xt_g[kt // 4][:, jc, (kt % 4) * P:(kt % 4 + 1) * P],
                            qt_t[:, jc, co:512],
                            start=(jc == 0), stop=(jc == DC - 1))
                    mk_t = p_mk.tile([P, w], F32, tag="mk")
                    if joint:
                        nc.vector.tensor_scalar(
                            mk_t[:, 0:QB], iota_t[:],
                            thr_t[:, baseA + kt:baseA + kt + 1], MASK_NEG,
                            op0=mybir.AluOpType.is_lt,
                            op1=mybir.AluOpType.mult)
                    nc.vector.tensor_scalar(
                        mk_t[:, (QB - co):w], iota_t[:],
                        thr_t[:, baseB + kt:baseB + kt + 1], MASK_NEG,
                        op0=mybir.AluOpType.is_lt, op1=mybir.AluOpType.mult)
                    sm_t = p_sm.tile([P, w], F32, tag="sm")
                    nc.vector.tensor_add(sm_t[:], acc[:], mk_t[:])
                    nc.scalar.activation(e_t[:, kt, co:512], sm_t[:],
                                         mybir.ActivationFunctionType.Exp,
                                         scale=SCALE)
                    anchor(("exp", bA, kt))
                    anchor(("exp", bB, kt))

                # U^T[d, q] = x^T E: joint accumulation; the shared-kt
                # chain covers both halves, the bB-only tiles continue
                # accumulating on the right half. Per-column-region
                # start/stop: left closes at kt nA-1, right at nB-1.
                us_t = p_us.tile([P, DC, 512], F16, tag="us")
                for dch in range(DC):
                    acc = ps_b.tile([P, 512], F32, tag="psb")
                    for kt in range(nB_):
                        lhs = xr_c[kt // 4][:, kt % 4, dch * P:(dch + 1) * P]
                        if kt < nA - 1:
                            nc.tensor.matmul(acc[:], lhs, e_t[:, kt, :],
                                             start=(kt == 0), stop=False)
                        elif kt == nA - 1:
                            nc.tensor.matmul(acc[:, 0:QB], lhs,
                                             e_t[:, kt, 0:QB],
                                             start=(kt == 0), stop=True)
                            nc.tensor.matmul(acc[:, QB:512], lhs,
                                             e_t[:, kt, QB:512],
                                             start=(kt == 0), stop=False)
                        else:
                            nc.tensor.matmul(acc[:, QB:512], lhs,
                                             e_t[:, kt, QB:512],
                                             start=False,
                                             stop=(kt == nB_ - 1))
                    nc.scalar.copy(us_t[:, dch, :], acc[:])
                    anchor(("us", bA, dch))

                # denominators d[q] = sum_k E[k, q] (per block)
                dinv = []
                for qs in range(4):
                    blk_n = nA if qs < 2 else nB_
                    d_acc = psd.tile([P, 8], F32, tag="d")
                    for kt in range(blk_n):
                        nc.tensor.matmul(d_acc[:, 0:2],
                                         e_t[:, kt, qs * P:(qs + 1) * P],
                                         ones_t[:],
                                         start=(kt == 0),
                                         stop=(kt == blk_n - 1))
                    dv = p_misc.tile([P, 1], F32, tag=f"dinv{pair}{qs}")
                    nc.vector.reciprocal(dv[:], d_acc[:, 0:1])
                    dinv.append(dv)

                # out[q, e] = (U Wv) / d
                for eh in range(2):
                    for qs in range(4):
                        acc = ps_b.tile([P, 512], F32, tag="psb")
                        for dc in range(DC):
                            nc.tensor.matmul(
                                acc[:],
                                us_t[:, dc, qs * P:(qs + 1) * P],
                                wv_h[eh][:, dc, :],
                                start=(dc == 0), stop=(dc == DC - 1))
                        o_t = p_out.tile([P, 512], F32, tag="o")
                        row = pair * 512 + qs * P
                        last = (pair == 1 and eh == 1 and qs >= 2)
                        if not last:
                            nc.vector.tensor_scalar_mul(o_t[:], acc[:],
                                                        dinv[qs][:])
                            nc.sync.dma_start(
                                out_d.ap()[eh][row:row + P, :], o_t[:])
                        else:
                            # split the very last tile so its writeback
                            # pipelines instead of sitting on the tail
                            for hh in range(2):
                                sl = slice(hh * 256, (hh + 1) * 256)
                                nc.vector.tensor_scalar_mul(
                                    o_t[:, sl], acc[:, sl], dinv[qs][:])
                                nc.sync.dma_start(
                                    out_d.ap()[eh][row:row + P, sl],
                                    o_t[:, sl])

    nc.compile()
    return nc


def _prep_weights(Wq32, Wk32, Wv16):
    """Pre-arrange weights into SBUF tile layouts (shared by all cores)."""
    M16 = (Wq32 @ Wk32.T).astype(np.float16)               # [d, j]
    mm = np.ascontiguousarray(
        M16.reshape(DC, P, DC, P).transpose(1, 2, 0, 3))   # [p, jc, dc, j]
    wv = np.ascontiguousarray(
        Wv16.reshape(DC, P, 2, 512).transpose(2, 1, 0, 3))  # [eh, p, dc, e]
    return mm, wv


_IOTA = np.broadcast_to(
    np.arange(QB, dtype=np.float32), (P, QB)).copy()


def _prep_core_inputs(x16, xT16, mm, wv, b, h):
    """Host-side shard prep for core (batch b, half h)."""
    qlos = QLOS[h]
    tq = np.concatenate([np.arange(q, q + QB) for q in qlos])

    xTb = xT16[b]                                          # [D, T] fp16
    xq = np.ascontiguousarray(
        xTb[:, tq].reshape(DC, P, 2, 512).transpose(2, 1, 0, 3))
    xt = np.ascontiguousarray(
        xTb.reshape(DC, P, NG, 512).transpose(2, 1, 0, 3))
    xr = np.ascontiguousarray(
        x16[b].reshape(XRC, 4, P, D).transpose(0, 2, 1, 3))

    thr = np.empty((P, NKT), dtype=np.float32)
    base = 0
    for s in range(NB):
        for kt in range(SLOTS[s]):
            thr[:, base + kt] = kt * P + np.arange(P) - qlos[s]
        base += SLOTS[s]

    return {
        "xq": xq, "mm": mm, "xt": xt, "xr": xr, "wv": wv,
        "iota": _IOTA, "thr": thr,
    }, tq


def kernel(x, Wq, Wk, Wv):
    global last_exec_time_ns
    x = np.asarray(x, dtype=np.float32)
    assert x.shape == (B, T, D)

    if "nc" not in _CACHE:
        _CACHE["nc"] = _build_program()
    nc = _CACHE["nc"]

    x16 = x.astype(np.float16)
    xT16 = np.ascontiguousarray(x16.transpose(0, 2, 1))    # [B, D, T]
    mm, wv = _prep_weights(
        np.asarray(Wq, dtype=np.float32),
        np.asarray(Wk, dtype=np.float32),
        np.asarray(Wv, dtype=np.float16))

    in_maps = []
    row_maps = []
    for c in range(8):
        im, tq = _prep_core_inputs(x16, xT16, mm, wv, c // 2, c % 2)
        in_maps.append(im)
        row_maps.append(tq)

    trace = bool(os.environ.get("BASS_KERNEL_TRACE"))
    kw = {}
    if trace:
        kw = {"trace": True, "tmpdir": os.environ.get(
            "BASS_KERNEL_TRACE_DIR", "/tmp/kernel_trace")}
    res = run_bass_kernel_spmd(nc, in_maps, core_ids=list(range(8)), **kw)
    if trace:
        last_exec_time_ns = res.exec_time_ns

    out = np.empty((B, T, D), dtype=np.float32)
    for c in range(8):
        o = res.results[c]["out"]                          # [2, NQ, 512]
        out[c // 2, row_maps[c]] = o.transpose(1, 0, 2).reshape(NQ, D)
    return out


# revision 40
# speedup vs baseline: 1.0737x; 1.0737x over previous
"""Causal single-head attention on 8 Trainium2 NeuronCores.

Problem: x [4, 2048, 1024] f32; Wq/Wk/Wv [1024, 1024] f32.
  q,k,v = x@W*; out = softmax(causal(q k^T / sqrt(d))) @ v.

Two algebraic folds remove all cross-core communication:

1. scores = (x Wq)(x Wk)^T = x (Wq Wk^T) x^T. The host precomputes
   M = Wq Wk^T once (weight-only preprocessing), so the device computes
   q~ = x M and uses raw x^T as the key matrix — the whole k projection
   and any k exchange disappear.
2. att @ v = att @ (x Wv) = ((x^T E)^T Wv): the device computes
   U^T[d, q] = x^T E (contracting keys) and then out = (U Wv)/denom.
   Same tensor-engine row count as v-proj + att@v, but v never needs to
   be materialized. The kernel has NO collectives; every operand is a
   direct per-core input.

Sharding: 8 cores = 4 batches x 2 query-interleavings. Each core owns
four 256-query blocks chosen so the causal key-prefix lengths fit the
shared ascending slot shape (4, 8, 12, 16) x 128-key tiles with only 4
padded tiles per core (exact fold balance; SPMD: one program, all
cores). Blocks run smallest-first so the first block needs the least
input data, and are processed as PAIRS (0,1) and (2,3): the pair's two
query blocks sit contiguously in qt/us, so scores and U^T run as joint
N=512 chains over the pair's shared key tiles and narrow N=256 chains
for the larger block's extra tiles (wider matmuls stream ~8% more
rows/cycle and halve instruction count).

Causal masking is generated ON DEVICE: one iota row constant plus a
per-tile threshold column (~13 KB total DMA) expand to the additive
-60000 mask via a fused vector compare-multiply, replacing 3.1 MB of
host mask DMA that would rate-limit the scores phase.

DMA scheduling: HBM bandwidth is shared fairly per inflight TRANSFER,
so issue order is everything. Wave 1 (issued at program start, 5 small
transfers across the sync/scalar/gpsimd queues) carries only the
q~-projection working set; every other input is issued by a dma_start
anchored behind a specific scalar-engine compute op, so it cannot
enter flight early and steal bandwidth from earlier-needed bytes. Each
anchored DMA must still precede its first consumer in PROGRAM order —
an anchor after the consumer inverts the dependency into
read-before-write garbage.

All matmul operands are fp16 (1 PE cycle/row; fp32 is 4x slower).
Accumulation stays fp32 in PSUM throughout. PSUM start/stop flags are
managed per column REGION of the accumulator (legal: regions are
independent element-wise RMW state), which lets the joint U^T chain
close its left half at the small block's last key tile while the right
half keeps accumulating.

Per-core dataflow per block pair (joint width 512, solo width 256):
  q~T[j, q] = M^T x_q^T               (one 512-wide pass)
  scores S^T[k, q] = x-block q~T      (accum over 8 j-chunks)
  E = exp((S^T + mask) / 32)          (ACT, fp16 out)
  U^T[d, q] = x-rows^T E              (accum over key tiles)
  d[q] = E^T 1                        (N=2 ones-matmul per block)
  out[q, e] = (U^T^T Wv) * (1/d)      (accum over 8 d-chunks)

Softmax max-subtraction is skipped deliberately: logits*scale are
bounded, so exp is well-conditioned.
"""

import os
import sys
from contextlib import ExitStack

sys.path.insert(0, "/opt/trn_rl_repo")

import numpy as np

import concourse.bass as bass  # noqa: F401
import concourse.tile as tile
from concourse import bacc, mybir
from concourse.bass_utils import run_bass_kernel_spmd

B, T, D = 4, 2048, 1024
P = 128                 # partitions
DC = D // P             # 8 contraction chunks
NQ = 1024               # queries per core
NKTS = (8, 16)          # key tiles per group
TCH = T // P            # 16 key 128-token chunks
XRC = 4                 # x-row DMA chunks (4 key tiles each)
NG = 4                  # x^T DMA groups (4 key tiles each)
SCALE = 1.0 / 32.0      # 1/sqrt(D)
MASK_NEG = -60000.0


def WIDTH(g, kt):
    """Joint matmul width at key tile kt of group g: four 128-query
    slices, sorted by descending causal prefix, drop out pairwise as kt
    passes their prefix ends. Compile-time, shared by all cores."""
    if g == 0:
        return 512 - P * (kt // 2)
    return 512 if kt < 10 else 512 - P * ((kt - 8) // 2)


# kts for which slice s of group g is computed (= its d-chain length)
LIVE = tuple(
    tuple(sum(1 for kt in range(NKTS[g]) if WIDTH(g, kt) >= (s + 1) * P)
          for s in range(4))
    for g in range(2))

# flat threshold-column index per (g, kt, s)
TIDX = {}
_i = 0
for _g in range(2):
    for _kt in range(NKTS[_g]):
        for _s in range(WIDTH(_g, _kt) // P):
            TIDX[(_g, _kt, _s)] = _i
            _i += 1
NTHR = _i  # 72

# query-slice starts per (half, group, slice); slices sorted by
# descending causal count so the width profile covers every core
QLOS = (((896, 640, 384, 128), (1920, 1664, 1408, 1152)),   # even cores
        ((768, 512, 256, 0), (1792, 1536, 1280, 1024)))     # odd cores

F16 = mybir.dt.float16
F32 = mybir.dt.float32

_CACHE = {}

last_exec_time_ns = None  # set when BASS_KERNEL_TRACE=1


def _build_program():
    nc = bacc.Bacc("TRN2", target_bir_lowering=False, debug=False, num_devices=8)

    xq_d = nc.dram_tensor("xq", [2, P, DC, 512], F16, kind="ExternalInput")
    mm_d = nc.dram_tensor("mm", [P, DC, DC, P], F16, kind="ExternalInput")
    xt_d = nc.dram_tensor("xt", [NG, P, DC, 512], F16, kind="ExternalInput")
    xr_d = nc.dram_tensor("xr", [XRC, P, 4, D], F16, kind="ExternalInput")
    wv_d = nc.dram_tensor("wv", [2, P, DC, 512], F16, kind="ExternalInput")
    iota_d = nc.dram_tensor("iota", [P, P], F32, kind="ExternalInput")
    thr_d = nc.dram_tensor("thr", [P, NTHR], F32, kind="ExternalInput")
    out_d = nc.dram_tensor("out", [2, NQ, 512], F32, kind="ExternalOutput")

    with tile.TileContext(nc) as tc, ExitStack() as stack:
        p_mm = stack.enter_context(tc.tile_pool(name="mm", bufs=1))
        p_xq = stack.enter_context(tc.tile_pool(name="xq", bufs=1))
        p_xt = stack.enter_context(tc.tile_pool(name="xt", bufs=1))
        p_xr = stack.enter_context(tc.tile_pool(name="xr", bufs=1))
        p_wv = stack.enter_context(tc.tile_pool(name="wv", bufs=1))
        p_qt = stack.enter_context(tc.tile_pool(name="qt", bufs=2))
        p_us = stack.enter_context(tc.tile_pool(name="us", bufs=2))
        p_e = stack.enter_context(tc.tile_pool(name="e", bufs=1))
        p_misc = stack.enter_context(tc.tile_pool(name="misc", bufs=1))
        p_mk = stack.enter_context(tc.tile_pool(name="mk", bufs=2))
        p_sm = stack.enter_context(tc.tile_pool(name="sm", bufs=2))
        p_out = stack.enter_context(tc.tile_pool(name="outp", bufs=3))
        ps_a = stack.enter_context(tc.tile_pool(name="psa", bufs=2, space="PSUM"))
        ps_b = stack.enter_context(tc.tile_pool(name="psb", bufs=3, space="PSUM"))
        psd = stack.enter_context(tc.tile_pool(name="psd", bufs=2, space="PSUM"))
        if True:
            # ---- tiny constants (scalar queue, ahead of everything) ----
            iota_t = p_misc.tile([P, P], F32, tag="iota")
            nc.scalar.dma_start(iota_t[:], iota_d.ap())
            thr_t = p_misc.tile([P, NTHR], F32, tag="thr")
            nc.scalar.dma_start(thr_t[:], thr_d.ap())
            ones_t = p_misc.tile([P, 2], F16, tag="ones")
            nc.vector.memset(ones_t[:], 1.0)

            # ---- input loads: explicit need-ordered queue assignment.
            xt_g = []
            xr_c = [None] * XRC

            # Allocate all input tiles up front; DMA issue is staged.
            xqA = p_xq.tile([P, DC, 512], F16, tag="xq0")
            xqB = p_xq.tile([P, DC, 512], F16, tag="xq1")
            xq_p = [xqA, xqB]
            mm_t = p_mm.tile([P, DC, DC, P], F16, tag="mm")
            for g in range(NG):
                xg = p_xt.tile([P, DC, 512], F16, tag=f"xt{g}")
                xt_g.append(xg)
            for c in range(XRC):
                xc = p_xr.tile([P, 4, D], F16, tag=f"xr{c}")
                xr_c[c] = xc
            wv0_t = p_wv.tile([P, DC, 512], F16, tag="wv0")
            wv1_t = p_wv.tile([P, DC, 512], F16, tag="wv1")
            wv_h = [wv0_t, wv1_t]

            # Immediate issues — ONLY the q~ pair-A working set. HBM
            # splits bandwidth per inflight TRANSFER, so the first wave
            # must be as few transfers as possible (5 here) for the
            # first-needed bytes to land fast.
            nc.sync.dma_start(xqA[:, 0:4], xq_d.ap()[0][:, 0:4])
            nc.gpsimd.dma_start(xqA[:, 4:8], xq_d.ap()[0][:, 4:8])
            nc.scalar.dma_start(mm_t[:, 0:2], mm_d.ap()[:, 0:2])
            nc.scalar.dma_start(mm_t[:, 2:4], mm_d.ap()[:, 2:4])
            nc.scalar.dma_start(mm_t[:, 4:8], mm_d.ap()[:, 4:8])

            # Deferred issues, anchored behind scalar compute ops so
            # they cannot enter flight before the anchor executes —
            # each wave stays small and lands well before its consumer.
            defer = {
                ("qt", 0, 0): (xt_g[0][:], xt_d.ap()[0]),
                ("qt", 0, 2): (xt_g[1][:], xt_d.ap()[1]),
                ("qt", 0, 4): (xr_c[0][:], xr_d.ap()[0]),
                ("qt", 0, 6): (xr_c[1][:], xr_d.ap()[1]),
                ("exp", 0, 1): (wv0_t[:], wv_d.ap()[0]),
                ("exp", 0, 3): (wv1_t[:], wv_d.ap()[1]),
                ("us", 0, 3): (xqB[:], xq_d.ap()[1]),
                ("us", 0, 6): (xt_g[2][:], xt_d.ap()[2]),
                ("qt", 1, 2): (xt_g[3][:], xt_d.ap()[3]),
                ("qt", 1, 5): (xr_c[2][:], xr_d.ap()[2]),
                ("exp", 1, 2): (xr_c[3][:], xr_d.ap()[3]),
            }

            def anchor(key):
                if key in defer:
                    dst, src = defer.pop(key)
                    nc.scalar.dma_start(dst, src)

            # ---- per-group pipeline: each group holds four 128-query
            # slices (descending causal prefix) contiguous in qt/us, so
            # scores and U^T run as width-stepped joint chains
            # (512 -> 384 -> 256 -> 128 as slices' prefixes end).
            for g in range(2):
                nkt = NKTS[g]

                # q~ projection, one 512-wide pass per group
                qt_t = p_qt.tile([P, DC, 512], F16, tag="qt")
                for jc in range(DC):
                    acc = ps_b.tile([P, 512], F32, tag="psb")
                    for dc in range(DC):
                        nc.tensor.matmul(acc[:],
                                         mm_t[:, jc, dc, :],
                                         xq_p[g][:, dc, :],
                                         start=(dc == 0),
                                         stop=(dc == DC - 1))
                    nc.scalar.copy(qt_t[:, jc, :], acc[:])
                    anchor(("qt", g, jc))

                # scores + on-device causal mask + exp, joint width per
                # key tile; per-slice thresholds restore causality
                # (including dead-slice padding on odd cores).
                e_t = p_e.tile([P, nkt, 512], F16, tag=f"e{g}")
                for kt in range(nkt):
                    w = WIDTH(g, kt)
                    acc = ps_b.tile([P, w], F32, tag="psb")
                    for jc in range(DC):
                        nc.tensor.matmul(
                            acc[:],
                            xt_g[kt // 4][:, jc, (kt % 4) * P:(kt % 4 + 1) * P],
                            qt_t[:, jc, 0:w],
                            start=(jc == 0), stop=(jc == DC - 1))
                    mk_t = p_mk.tile([P, w], F32, tag="mk")
                    for s in range(w // P):
                        ti = TIDX[(g, kt, s)]
                        nc.vector.tensor_scalar(
                            mk_t[:, s * P:(s + 1) * P], iota_t[:],
                            thr_t[:, ti:ti + 1], MASK_NEG,
                            op0=mybir.AluOpType.is_lt,
                            op1=mybir.AluOpType.mult)
                    sm_t = p_sm.tile([P, w], F32, tag="sm")
                    nc.vector.tensor_add(sm_t[:], acc[:], mk_t[:])
                    nc.scalar.activation(e_t[:, kt, 0:w], sm_t[:],
                                         mybir.ActivationFunctionType.Exp,
                                         scale=SCALE)
                    anchor(("exp", g, kt))

                # U^T[d, q] = x^T E, width-stepped accumulation: at each
                # width drop the vacated column region gets its closing
                # stop while the remainder keeps accumulating (regions
                # are independent element-wise PSUM state).
                us_t = p_us.tile([P, DC, 512], F16, tag="us")
                for dch in range(DC):
                    acc = ps_b.tile([P, 512], F32, tag="psb")
                    for kt in range(nkt):
                        w = WIDTH(g, kt)
                        wn = WIDTH(g, kt + 1) if kt + 1 < nkt else 0
                        lhs = xr_c[kt // 4][:, kt % 4, dch * P:(dch + 1) * P]
                        if wn == w:
                            nc.tensor.matmul(acc[:, 0:w], lhs,
                                             e_t[:, kt, 0:w],
                                             start=(kt == 0), stop=False)
                        else:
                            if wn > 0:
                                nc.tensor.matmul(acc[:, 0:wn], lhs,
                                                 e_t[:, kt, 0:wn],
                                                 start=(kt == 0),
                                                 stop=False)
                            nc.tensor.matmul(acc[:, wn:w], lhs,
                                             e_t[:, kt, wn:w],
                                             start=(kt == 0), stop=True)
                    nc.scalar.copy(us_t[:, dch, :], acc[:])
                    anchor(("us", g, dch))

                # denominators d[q] = sum_k E[k, q] (per slice)
                dinv = []
                for s in range(4):
                    blk_n = LIVE[g][s]
                    d_acc = psd.tile([P, 8], F32, tag="d")
                    for kt in range(blk_n):
                        nc.tensor.matmul(d_acc[:, 0:2],
                                         e_t[:, kt, s * P:(s + 1) * P],
                                         ones_t[:],
                                         start=(kt == 0),
                                         stop=(kt == blk_n - 1))
                    dv = p_misc.tile([P, 1], F32, tag=f"dinv{g}{s}")
                    nc.vector.reciprocal(dv[:], d_acc[:, 0:1])
                    dinv.append(dv)

                # out[q, e] = (U Wv) / d
                for eh in range(2):
                    for s in range(4):
                        acc = ps_b.tile([P, 512], F32, tag="psb")
                        for dc in range(DC):
                            nc.tensor.matmul(
                                acc[:],
                                us_t[:, dc, s * P:(s + 1) * P],
                                wv_h[eh][:, dc, :],
                                start=(dc == 0), stop=(dc == DC - 1))
                        o_t = p_out.tile([P, 512], F32, tag="o")
                        row = g * 512 + s * P
                        last = (g == 1 and eh == 1 and s >= 2)
                        if not last:
                            nc.vector.tensor_scalar_mul(o_t[:], acc[:],
                                                        dinv[s][:])
                            nc.sync.dma_start(
                                out_d.ap()[eh][row:row + P, :], o_t[:])
                        else:
                            # split the very last tiles so writeback
                            # pipelines instead of sitting on the tail
                            for hh in range(2):
                                sl = slice(hh * 256, (hh + 1) * 256)
                                nc.vector.tensor_scalar_mul(
                                    o_t[:, sl], acc[:, sl], dinv[s][:])
                                nc.sync.dma_start(
                                    out_d.ap()[eh][row:row + P, sl],
                                    o_t[:, sl])

    nc.compile()
    return nc


def _prep_weights(Wq32, Wk32, Wv16):
    """Pre-arrange weights into SBUF tile layouts (shared by all cores)."""
    M16 = (Wq32 @ Wk32.T).astype(np.float16)               # [d, j]
    mm = np.ascontiguousarray(
        M16.reshape(DC, P, DC, P).transpose(1, 2, 0, 3))   # [p, jc, dc, j]
    wv = np.ascontiguousarray(
        Wv16.reshape(DC, P, 2, 512).transpose(2, 1, 0, 3))  # [eh, p, dc, e]
    return mm, wv


_IOTA = np.broadcast_to(
    np.arange(P, dtype=np.float32), (P, P)).copy()


def _prep_core_inputs(x16, xT16, mm, wv, b, h):
    """Host-side shard prep for core (batch b, half h)."""
    qlos = QLOS[h]
    tq = np.concatenate([np.arange(q, q + P)
                         for g in range(2) for q in qlos[g]])

    xTb = xT16[b]                                          # [D, T] fp16
    xq = np.ascontiguousarray(
        xTb[:, tq].reshape(DC, P, 2, 512).transpose(2, 1, 0, 3))
    xt = np.ascontiguousarray(
        xTb.reshape(DC, P, NG, 512).transpose(2, 1, 0, 3))
    xr = np.ascontiguousarray(
        x16[b].reshape(XRC, 4, P, D).transpose(0, 2, 1, 3))

    thr = np.empty((P, NTHR), dtype=np.float32)
    for (g, kt, s), ti in TIDX.items():
        thr[:, ti] = kt * P + np.arange(P) - qlos[g][s]

    return {
        "xq": xq, "mm": mm, "xt": xt, "xr": xr, "wv": wv,
        "iota": _IOTA, "thr": thr,
    }, tq


def kernel(x, Wq, Wk, Wv):
    global last_exec_time_ns
    x = np.asarray(x, dtype=np.float32)
    assert x.shape == (B, T, D)

    if "nc" not in _CACHE:
        _CACHE["nc"] = _build_program()
    nc = _CACHE["nc"]

    x16 = x.astype(np.float16)
    xT16 = np.ascontiguousarray(x16.transpose(0, 2, 1))    # [B, D, T]
    mm, wv = _prep_weights(
        np.asarray(Wq, dtype=np.float32),
        np.asarray(Wk, dtype=np.float32),
        np.asarray(Wv, dtype=np.float16))

    in_maps = []
    row_maps = []
    for c in range(8):
        im, tq = _prep_core_inputs(x16, xT16, mm, wv, c // 2, c % 2)
        in_maps.append(im)
        row_maps.append(tq)

    trace = bool(os.environ.get("BASS_KERNEL_TRACE"))
    kw = {}
    if trace:
        kw = {"trace": True, "tmpdir": os.environ.get(
            "BASS_KERNEL_TRACE_DIR", "/tmp/kernel_trace")}
    res = run_bass_kernel_spmd(nc, in_maps, core_ids=list(range(8)), **kw)
    if trace:
        last_exec_time_ns = res.exec_time_ns

    out = np.empty((B, T, D), dtype=np.float32)
    for c in range(8):
        o = res.results[c]["out"]                          # [2, NQ, 512]
        out[c // 2, row_maps[c]] = o.transpose(1, 0, 2).reshape(NQ, D)
    return out
